# revision 6
# baseline (speedup 1.0000x reference)
"""MultiHeadAttention kernel for 8 Trainium2 NeuronCores.

Problem: B=4, S=2048, D=1024, H=16, dk=64. mask in {0,1}.
reference returns (output[B,S,D], attn[B,H,S,S]).

Sharding: core c -> batch b = c//2, head-group g = c%2 (8 heads each).
Megatron-style: Wq/Wk/Wv column-parallel (head split), Wo row-parallel;
the 2-way partial-sum reduce of the output projection and the +bo bias
are done on host (cheap: 8MB/core).

Per-core compute layout (everything "transposed"):
  QT[d'=512, S]  = (wq_g/8 @ q_b^T) + bq_g/8      (rank-1 bias matmul)
  KT[d'=512, S]  = wk_g @ k_b^T + bk_g
  V1[S, 8*65]    = v_b @ wv_g^T + bv_g, heads interleaved with a ones
                   column per head (rowsum trick)
  per head h, sq-stripe j (512 wide):
    T^T[sk,sq]   = Kh @ Qh^T                      (16 matmuls [64,128]x[64,512])
    E^T          = exp(T^T) * mask_b^T            (ACT exp; DVE mul)
    out'[65,512] = V1h'^T @ E^T                   (16 accum matmuls; row 64 = rowsum)
    recip        = 1/out'[64]; bcast across partitions (gpsimd)
    attnT[h]     = E^T * bcast                    -> DMA out (host transposes)
    combT        = out'[0:64] * bcast             (f32r, kept in SBUF)
  outp[S, D]     = combT^T @ wo_g^T               (natural layout) -> DMA out

Matmuls run in float32r (tf32-like, ~3e-4 rounding) at full PE speed.
"""
import sys

if "/opt/trn_rl_repo" not in sys.path:
    sys.path.insert(0, "/opt/trn_rl_repo")

import os
import numpy as np
from concurrent.futures import ThreadPoolExecutor

import concourse.bass as bass
import concourse.mybir as mybir
import concourse.tile as tile
from concourse import bacc
from concourse.bass_utils import run_bass_kernel_spmd

F32 = mybir.dt.float32
F32R = mybir.dt.float32r
EXP = mybir.ActivationFunctionType.Exp

S = 2048          # sequence length
D = 1024          # model dim
DKH = 512         # per-core head-group width (8 heads * 64)
NH = 8            # heads per core
DK = 64
SP = 512          # phase-2 sq stripe width
NJ = S // SP      # 4 stripes
SKT = S // 128    # 16 sk tiles
NKT = D // 128    # 8 contraction tiles for projections

_CACHED_NC = [None]


def _build_nc():
    nc = bacc.Bacc("TRN2", target_bir_lowering=False, debug=False, num_devices=8)

    qT = nc.dram_tensor("qT", [D, S], F32R, kind="ExternalInput")
    kT = nc.dram_tensor("kT", [D, S], F32R, kind="ExternalInput")
    vT = nc.dram_tensor("vT", [D, S], F32R, kind="ExternalInput")
    maskT = nc.dram_tensor("maskT", [S, S], mybir.dt.bfloat16, kind="ExternalInput")
    wqT = nc.dram_tensor("wqT", [D, DKH], F32R, kind="ExternalInput")
    wkT = nc.dram_tensor("wkT", [D, DKH], F32R, kind="ExternalInput")
    wvT = nc.dram_tensor("wvT", [D, DKH], F32R, kind="ExternalInput")
    woT = nc.dram_tensor("woT", [DKH, D], F32R, kind="ExternalInput")
    bq = nc.dram_tensor("bq", [1, DKH], F32R, kind="ExternalInput")
    bk = nc.dram_tensor("bk", [1, DKH], F32R, kind="ExternalInput")
    bv = nc.dram_tensor("bv", [1, DKH], F32R, kind="ExternalInput")
    ones_d = nc.dram_tensor("ones_d", [128, 512], F32R, kind="ExternalInput")

    attnT = nc.dram_tensor("attnT", [NH, S, S], F32, kind="ExternalOutput")
    outp = nc.dram_tensor("outp", [S, D], F32, kind="ExternalOutput")

    with tile.TileContext(nc) as tc:
        with tc.tile_pool(name="persist", bufs=1) as persist, \
             tc.tile_pool(name="singles", bufs=1) as singles:
            combT = [persist.tile([128, S], F32R, name=f"combT{t}", tag=f"combT{t}")
                     for t in range(4)]
            ones_t = singles.tile([128, 512], F32R, name="ones_t")
            nc.sync.dma_start(out=ones_t, in_=ones_d[:, :])

            with tc.tile_pool(name="qkv", bufs=1) as qkv:
                QT = [qkv.tile([128, S], F32R, name=f"QT{t}", tag=f"QT{t}")
                      for t in range(4)]
                KT = [qkv.tile([128, S], F32R, name=f"KT{t}", tag=f"KT{t}")
                      for t in range(4)]
                V1 = [qkv.tile([128, NH * 65], F32R, name=f"V1_{t}", tag=f"V1_{t}")
                      for t in range(SKT)]

                # ---------------- Phase 1: projections ----------------
                with tc.tile_pool(name="ph1w", bufs=1) as ph1w, \
                     tc.tile_pool(name="ph1in", bufs=2) as ph1in, \
                     tc.tile_pool(name="ph1b", bufs=1) as ph1b, \
                     tc.tile_pool(name="ph1ps", bufs=2, space="PSUM") as ph1ps:

                    # Q^T and K^T projections: OUT[m][:, n] = w^T-tiles.T @ x^T + b
                    for pname, w_d, b_d, OUT, x_d in (
                        ("q", wqT, bq, QT, qT),
                        ("k", wkT, bk, KT, kT),
                    ):
                        b_t = ph1b.tile([1, DKH], F32R, name=f"b_{pname}",
                                        tag=f"b_{pname}")
                        nc.sync.dma_start(out=b_t, in_=b_d[:, :])
                        w_sb = []
                        for k in range(NKT):
                            wt = ph1w.tile([128, DKH], F32R,
                                           name=f"w{pname}_{k}", tag=f"w_{k}")
                            nc.sync.dma_start(
                                out=wt, in_=w_d[k * 128:(k + 1) * 128, :])
                            w_sb.append(wt)
                        for n in range(4):
                            x_sb = []
                            for k in range(NKT):
                                xt = ph1in.tile([128, 512], F32R,
                                                name=f"x{pname}_{k}_{n}",
                                                tag=f"x_{k}")
                                nc.sync.dma_start(
                                    out=xt,
                                    in_=x_d[k * 128:(k + 1) * 128,
                                            n * 512:(n + 1) * 512])
                                x_sb.append(xt)
                            for m in range(4):
                                ps = ph1ps.tile([128, 512], F32,
                                                name=f"ps{pname}_{m}_{n}",
                                                tag="ps_proj")
                                nc.tensor.matmul(
                                    ps, b_t[0:1, m * 128:(m + 1) * 128],
                                    ones_t[0:1, :],
                                    start=True, stop=False)
                                for k in range(NKT):
                                    nc.tensor.matmul(
                                        ps,
                                        w_sb[k][:, m * 128:(m + 1) * 128],
                                        x_sb[k],
                                        start=False, stop=(k == NKT - 1))
                                nc.vector.tensor_copy(
                                    OUT[m][:, n * 512:(n + 1) * 512], ps)

                    # V projection (natural layout), heads padded with ones col
                    b_t = ph1b.tile([1, DKH], F32R, name="b_v", tag="b_v")
                    nc.sync.dma_start(out=b_t, in_=bv[:, :])
                    w_sb = []
                    for k in range(NKT):
                        wt = ph1w.tile([128, DKH], F32R,
                                       name=f"wv_{k}", tag=f"w_{k}")
                        nc.sync.dma_start(
                            out=wt, in_=wvT[k * 128:(k + 1) * 128, :])
                        w_sb.append(wt)
                    for n in range(4):
                        x_sb = []
                        for k in range(NKT):
                            xt = ph1in.tile([128, 512], F32R,
                                            name=f"xv_{k}_{n}", tag=f"x_{k}")
                            nc.sync.dma_start(
                                out=xt,
                                in_=vT[k * 128:(k + 1) * 128,
                                       n * 512:(n + 1) * 512])
                            x_sb.append(xt)
                        for sl in range(4):
                            s = n * 4 + sl
                            ps = ph1ps.tile([128, 512], F32,
                                            name=f"psv_{s}", tag="ps_proj")
                            nc.tensor.matmul(
                                ps, ones_t[0:1, 0:128], b_t[0:1, :],
                                start=True, stop=False)
                            for k in range(NKT):
                                nc.tensor.matmul(
                                    ps,
                                    x_sb[k][:, sl * 128:(sl + 1) * 128],
                                    w_sb[k],
                                    start=False, stop=(k == NKT - 1))
                            for h in range(NH):
                                nc.vector.tensor_copy(
                                    V1[s][:, h * 65:h * 65 + 64],
                                    ps[:, h * 64:(h + 1) * 64])
                                nc.vector.tensor_copy(
                                    V1[s][:, h * 65 + 64:h * 65 + 65],
                                    ones_t[:, 0:1])

                # ---------------- Phase 2: attention ----------------
                with tc.tile_pool(name="ph2m", bufs=1) as ph2m, \
                     tc.tile_pool(name="ph2er", bufs=3) as ph2er, \
                     tc.tile_pool(name="ph2em", bufs=1) as ph2em, \
                     tc.tile_pool(name="ph2at", bufs=3) as ph2at, \
                     tc.tile_pool(name="ph2bc", bufs=3) as ph2bc, \
                     tc.tile_pool(name="ph2ps", bufs=4, space="PSUM") as ph2ps, \
                     tc.tile_pool(name="ph2pav", bufs=2, space="PSUM") as ph2pav:
                    for j in range(NJ):
                        m_sb = []
                        for i in range(SKT):
                            mt = ph2m.tile([128, SP], mybir.dt.bfloat16,
                                           name=f"m_{i}_{j}", tag=f"m_{i}")
                            nc.sync.dma_start(
                                out=mt,
                                in_=maskT[i * 128:(i + 1) * 128,
                                          j * SP:(j + 1) * SP])
                            m_sb.append(mt)
                        for h in range(NH):
                            ht, ho = h // 2, (h % 2) * 64
                            em_sb = []
                            for i in range(SKT):
                                ps = ph2ps.tile([128, SP], F32,
                                                name=f"pss_{h}_{i}_{j}",
                                                tag="ps_s")
                                nc.tensor.matmul(
                                    ps,
                                    KT[ht][ho:ho + 64, i * 128:(i + 1) * 128],
                                    QT[ht][ho:ho + 64, j * SP:(j + 1) * SP],
                                    start=True, stop=True)
                                er = ph2er.tile([128, SP], F32,
                                                name=f"er_{h}_{i}_{j}",
                                                tag="er")
                                nc.scalar.activation(out=er, in_=ps, func=EXP)
                                em = ph2em.tile([128, SP], F32R,
                                                name=f"em_{h}_{i}_{j}",
                                                tag=f"em_{i}")
                                nc.vector.tensor_mul(em, er, m_sb[i])
                                em_sb.append(em)
                            av = ph2pav.tile([128, SP], F32,
                                             name=f"av_{h}_{j}", tag="ps_av")
                            for i in range(SKT):
                                nc.tensor.matmul(
                                    av[0:65, :],
                                    V1[i][:, h * 65:(h + 1) * 65],
                                    em_sb[i],
                                    start=(i == 0), stop=(i == SKT - 1))
                            rec = ph2bc.tile([1, SP], F32,
                                             name=f"rec_{h}_{j}", tag="rec")
                            nc.vector.reciprocal(rec, av[64:65, :])
                            bc = ph2bc.tile([128, SP], F32,
                                            name=f"bc_{h}_{j}", tag="bc")
                            nc.gpsimd.partition_broadcast(bc, rec)
                            for i in range(SKT):
                                at = ph2at.tile([128, SP], F32,
                                                name=f"at_{h}_{i}_{j}",
                                                tag="at")
                                nc.vector.tensor_mul(at, em_sb[i], bc)
                                nc.sync.dma_start(
                                    out=attnT[h, i * 128:(i + 1) * 128,
                                              j * SP:(j + 1) * SP],
                                    in_=at)
                            nc.vector.tensor_mul(
                                combT[ht][ho:ho + 64, j * SP:(j + 1) * SP],
                                av[0:64, :], bc[0:64, :])

            # ---------------- Phase 3: output projection ----------------
            with tc.tile_pool(name="ph3w", bufs=1) as ph3w, \
                 tc.tile_pool(name="ph3o", bufs=4) as ph3o, \
                 tc.tile_pool(name="ph3ps", bufs=2, space="PSUM") as ph3ps:
                wo_sb = []
                for k in range(4):
                    wt = ph3w.tile([128, D], F32R, name=f"wo_{k}", tag=f"wo_{k}")
                    nc.sync.dma_start(out=wt, in_=woT[k * 128:(k + 1) * 128, :])
                    wo_sb.append(wt)
                for s in range(SKT):
                    for n2 in range(2):
                        ps = ph3ps.tile([128, 512], F32,
                                        name=f"pso_{s}_{n2}", tag="ps_o")
                        for k in range(4):
                            nc.tensor.matmul(
                                ps,
                                combT[k][:, s * 128:(s + 1) * 128],
                                wo_sb[k][:, n2 * 512:(n2 + 1) * 512],
                                start=(k == 0), stop=(k == 3))
                        ot = ph3o.tile([128, 512], F32,
                                       name=f"ot_{s}_{n2}", tag="ot")
                        nc.vector.tensor_copy(ot, ps)
                        nc.sync.dma_start(
                            out=outp[s * 128:(s + 1) * 128,
                                     n2 * 512:(n2 + 1) * 512],
                            in_=ot)
    nc.compile()
    return nc


def kernel(q, k, v, mask, wq, wk, wv, wo, bq, bk, bv, bo):
    q = np.asarray(q, dtype=np.float32)
    k = np.asarray(k, dtype=np.float32)
    v = np.asarray(v, dtype=np.float32)
    mask = np.asarray(mask)
    wq = np.asarray(wq, dtype=np.float32)
    wk = np.asarray(wk, dtype=np.float32)
    wv = np.asarray(wv, dtype=np.float32)
    wo = np.asarray(wo, dtype=np.float32)
    bq = np.asarray(bq, dtype=np.float32)
    bk = np.asarray(bk, dtype=np.float32)
    bv = np.asarray(bv, dtype=np.float32)
    bo = np.asarray(bo, dtype=np.float32)

    if _CACHED_NC[0] is None:
        _CACHED_NC[0] = _build_nc()
    nc = _CACHED_NC[0]

    ones_arr = np.ones((128, 512), np.float32)
    B = q.shape[0]

    maskTs = {}
    qTs, kTs, vTs = {}, {}, {}
    def _prep_b(b):
        import ml_dtypes
        maskTs[b] = np.ascontiguousarray(mask[b].T).astype(ml_dtypes.bfloat16)
        qTs[b] = np.ascontiguousarray(q[b].T)
        kTs[b] = np.ascontiguousarray(k[b].T)
        vTs[b] = np.ascontiguousarray(v[b].T)
    with ThreadPoolExecutor(max_workers=8) as ex:
        list(ex.map(_prep_b, range(B)))

    in_maps = []
    for c in range(8):
        b, g = c // 2, c % 2
        gs = slice(g * DKH, (g + 1) * DKH)
        in_maps.append({
            "qT": qTs[b], "kT": kTs[b], "vT": vTs[b],
            "maskT": maskTs[b],
            "wqT": np.ascontiguousarray((wq[gs, :] / 8.0).T),
            "wkT": np.ascontiguousarray(wk[gs, :].T),
            "wvT": np.ascontiguousarray(wv[gs, :].T),
            "woT": np.ascontiguousarray(wo[:, gs].T),
            "bq": (bq[gs] / 8.0).reshape(1, DKH).copy(),
            "bk": bk[gs].reshape(1, DKH).copy(),
            "bv": bv[gs].reshape(1, DKH).copy(),
            "ones_d": ones_arr,
        })

    trace = os.environ.get("TRN_MHA_TRACE", "0") == "1"
    res = run_bass_kernel_spmd(nc, in_maps, core_ids=list(range(8)),
                               trace=trace)
    kernel._bass_results = res

    out = np.empty((B, S, D), np.float32)
    for b in range(B):
        out[b] = res.results[2 * b]["outp"] + res.results[2 * b + 1]["outp"] + bo

    attn = np.empty((B, 16, S, S), np.float32)
    def _tp(task):
        c, h = task
        b, g = c // 2, c % 2
        np.copyto(attn[b, g * NH + h], res.results[c]["attnT"][h].T)
    with ThreadPoolExecutor(max_workers=16) as ex:
        list(ex.map(_tp, [(c, h) for c in range(8) for h in range(NH)]))

    return out, attn


# revision 8
# speedup vs baseline: 1.1621x; 1.1621x over previous
"""MultiHeadAttention kernel for 8 Trainium2 NeuronCores.

Problem: B=4, S=2048, D=1024, H=16, dk=64. mask in {0,1}.
reference returns (output[B,S,D], attn[B,H,S,S]).

Sharding: core c -> batch b = c//2, head-group g = c%2 (8 heads each).
Megatron-style: Wq/Wk/Wv column-parallel (head split), Wo row-parallel;
the 2-way partial-sum reduce of the output projection and the +bo bias
are done on host (cheap: 8MB/core).

Per-core compute layout (everything "transposed"):
  QT[d'=512, S]  = (wq_g/8 @ q_b^T) + bq_g/8      (rank-1 bias matmul)
  KT[d'=512, S]  = wk_g @ k_b^T + bk_g
  V1[S, 8*65]    = v_b @ wv_g^T + bv_g, heads interleaved with a ones
                   column per head (rowsum trick)
  per head h, sq-stripe j (512 wide):
    T^T[sk,sq]   = Kh @ Qh^T                      (16 matmuls [64,128]x[64,512])
    E^T          = exp(T^T) * mask_b^T            (ACT exp; DVE mul)
    out'[65,512] = V1h'^T @ E^T                   (16 accum matmuls; row 64 = rowsum)
    recip        = 1/out'[64]; bcast across partitions (gpsimd)
    attnT[h]     = E^T * bcast                    -> DMA out (host transposes)
    combT        = out'[0:64] * bcast             (f32r, kept in SBUF)
  outp[S, D]     = combT^T @ wo_g^T               (natural layout) -> DMA out

Matmuls run in float32r (tf32-like, ~3e-4 rounding) at full PE speed.
"""
import sys

if "/opt/trn_rl_repo" not in sys.path:
    sys.path.insert(0, "/opt/trn_rl_repo")

import os
import numpy as np
from concurrent.futures import ThreadPoolExecutor

import concourse.bass as bass
import concourse.mybir as mybir
import concourse.tile as tile
from concourse import bacc
from concourse.bass_utils import run_bass_kernel_spmd

F32 = mybir.dt.float32
F32R = mybir.dt.float32r
EXP = mybir.ActivationFunctionType.Exp

S = 2048          # sequence length
D = 1024          # model dim
DKH = 512         # per-core head-group width (8 heads * 64)
NH = 8            # heads per core
DK = 64
SP = 512          # phase-2 sq stripe width
NJ = S // SP      # 4 stripes
SKT = S // 128    # 16 sk tiles
NKT = D // 128    # 8 contraction tiles for projections

_CACHED_NC = [None]


def _build_nc():
    nc = bacc.Bacc("TRN2", target_bir_lowering=False, debug=False, num_devices=8)

    qT = nc.dram_tensor("qT", [D, S], F32R, kind="ExternalInput")
    kT = nc.dram_tensor("kT", [D, S], F32R, kind="ExternalInput")
    vT = nc.dram_tensor("vT", [D, S], F32R, kind="ExternalInput")
    maskT = nc.dram_tensor("maskT", [S, S], mybir.dt.bfloat16, kind="ExternalInput")
    wqT = nc.dram_tensor("wqT", [D, DKH], F32R, kind="ExternalInput")
    wkT = nc.dram_tensor("wkT", [D, DKH], F32R, kind="ExternalInput")
    wvT = nc.dram_tensor("wvT", [D, DKH], F32R, kind="ExternalInput")
    woT = nc.dram_tensor("woT", [DKH, D], F32R, kind="ExternalInput")
    bq = nc.dram_tensor("bq", [1, DKH], F32R, kind="ExternalInput")
    bk = nc.dram_tensor("bk", [1, DKH], F32R, kind="ExternalInput")
    bv = nc.dram_tensor("bv", [1, DKH], F32R, kind="ExternalInput")
    ones_d = nc.dram_tensor("ones_d", [128, 512], F32R, kind="ExternalInput")

    attnT = nc.dram_tensor("attnT", [NH, S, S], F32, kind="ExternalOutput")
    rsums = nc.dram_tensor("rsums", [NH, S], F32, kind="ExternalOutput")
    outp = nc.dram_tensor("outp", [S, D], F32, kind="ExternalOutput")

    with tile.TileContext(nc) as tc:
        with tc.tile_pool(name="persist", bufs=1) as persist, \
             tc.tile_pool(name="singles", bufs=1) as singles:
            combT = [persist.tile([128, S], F32R, name=f"combT{t}", tag=f"combT{t}")
                     for t in range(4)]
            ones_t = singles.tile([128, 512], F32R, name="ones_t")
            nc.sync.dma_start(out=ones_t, in_=ones_d[:, :])

            with tc.tile_pool(name="qkv", bufs=1) as qkv:
                QT = [qkv.tile([128, S], F32R, name=f"QT{t}", tag=f"QT{t}")
                      for t in range(4)]
                KT = [qkv.tile([128, S], F32R, name=f"KT{t}", tag=f"KT{t}")
                      for t in range(4)]
                V1 = [qkv.tile([128, NH * 65], F32R, name=f"V1_{t}", tag=f"V1_{t}")
                      for t in range(SKT)]

                # ---------------- Phase 1: projections ----------------
                with tc.tile_pool(name="ph1w", bufs=1) as ph1w, \
                     tc.tile_pool(name="ph1in", bufs=2) as ph1in, \
                     tc.tile_pool(name="ph1b", bufs=1) as ph1b, \
                     tc.tile_pool(name="ph1ps", bufs=2, space="PSUM") as ph1ps:

                    # Q^T and K^T projections: OUT[m][:, n] = w^T-tiles.T @ x^T + b
                    for pname, w_d, b_d, OUT, x_d in (
                        ("q", wqT, bq, QT, qT),
                        ("k", wkT, bk, KT, kT),
                    ):
                        b_t = ph1b.tile([1, DKH], F32R, name=f"b_{pname}",
                                        tag=f"b_{pname}")
                        nc.sync.dma_start(out=b_t, in_=b_d[:, :])
                        w_sb = []
                        for k in range(NKT):
                            wt = ph1w.tile([128, DKH], F32R,
                                           name=f"w{pname}_{k}", tag=f"w_{k}")
                            nc.sync.dma_start(
                                out=wt, in_=w_d[k * 128:(k + 1) * 128, :])
                            w_sb.append(wt)
                        for n in range(4):
                            x_sb = []
                            for k in range(NKT):
                                xt = ph1in.tile([128, 512], F32R,
                                                name=f"x{pname}_{k}_{n}",
                                                tag=f"x_{k}")
                                nc.sync.dma_start(
                                    out=xt,
                                    in_=x_d[k * 128:(k + 1) * 128,
                                            n * 512:(n + 1) * 512])
                                x_sb.append(xt)
                            for m in range(4):
                                ps = ph1ps.tile([128, 512], F32,
                                                name=f"ps{pname}_{m}_{n}",
                                                tag="ps_proj")
                                nc.tensor.matmul(
                                    ps, b_t[0:1, m * 128:(m + 1) * 128],
                                    ones_t[0:1, :],
                                    start=True, stop=False)
                                for k in range(NKT):
                                    nc.tensor.matmul(
                                        ps,
                                        w_sb[k][:, m * 128:(m + 1) * 128],
                                        x_sb[k],
                                        start=False, stop=(k == NKT - 1))
                                nc.vector.tensor_copy(
                                    OUT[m][:, n * 512:(n + 1) * 512], ps)

                    # V projection (natural layout), heads padded with ones col
                    b_t = ph1b.tile([1, DKH], F32R, name="b_v", tag="b_v")
                    nc.sync.dma_start(out=b_t, in_=bv[:, :])
                    w_sb = []
                    for k in range(NKT):
                        wt = ph1w.tile([128, DKH], F32R,
                                       name=f"wv_{k}", tag=f"w_{k}")
                        nc.sync.dma_start(
                            out=wt, in_=wvT[k * 128:(k + 1) * 128, :])
                        w_sb.append(wt)
                    for n in range(4):
                        x_sb = []
                        for k in range(NKT):
                            xt = ph1in.tile([128, 512], F32R,
                                            name=f"xv_{k}_{n}", tag=f"x_{k}")
                            nc.sync.dma_start(
                                out=xt,
                                in_=vT[k * 128:(k + 1) * 128,
                                       n * 512:(n + 1) * 512])
                            x_sb.append(xt)
                        for sl in range(4):
                            s = n * 4 + sl
                            ps = ph1ps.tile([128, 512], F32,
                                            name=f"psv_{s}", tag="ps_proj")
                            nc.tensor.matmul(
                                ps, ones_t[0:1, 0:128], b_t[0:1, :],
                                start=True, stop=False)
                            for k in range(NKT):
                                nc.tensor.matmul(
                                    ps,
                                    x_sb[k][:, sl * 128:(sl + 1) * 128],
                                    w_sb[k],
                                    start=False, stop=(k == NKT - 1))
                            for h in range(NH):
                                nc.vector.tensor_copy(
                                    V1[s][:, h * 65:h * 65 + 64],
                                    ps[:, h * 64:(h + 1) * 64])
                                nc.vector.tensor_copy(
                                    V1[s][:, h * 65 + 64:h * 65 + 65],
                                    ones_t[:, 0:1])

                # ---------------- Phase 2: attention ----------------
                with tc.tile_pool(name="ph2m", bufs=1) as ph2m, \
                     tc.tile_pool(name="ph2er", bufs=4) as ph2er, \
                     tc.tile_pool(name="ph2em", bufs=1) as ph2em, \
                     tc.tile_pool(name="ph2bc", bufs=3) as ph2bc, \
                     tc.tile_pool(name="ph2ps", bufs=4, space="PSUM") as ph2ps, \
                     tc.tile_pool(name="ph2pav", bufs=2, space="PSUM") as ph2pav:
                    for j in range(NJ):
                        m_sb = []
                        for i in range(SKT):
                            mt = ph2m.tile([128, SP], mybir.dt.bfloat16,
                                           name=f"m_{i}_{j}", tag=f"m_{i}")
                            nc.sync.dma_start(
                                out=mt,
                                in_=maskT[i * 128:(i + 1) * 128,
                                          j * SP:(j + 1) * SP])
                            m_sb.append(mt)
                        for h in range(NH):
                            ht, ho = h // 2, (h % 2) * 64
                            em_sb = []
                            for i in range(SKT):
                                ps = ph2ps.tile([128, SP], F32,
                                                name=f"pss_{h}_{i}_{j}",
                                                tag="ps_s")
                                nc.tensor.matmul(
                                    ps,
                                    KT[ht][ho:ho + 64, i * 128:(i + 1) * 128],
                                    QT[ht][ho:ho + 64, j * SP:(j + 1) * SP],
                                    start=True, stop=True)
                                er = ph2er.tile([128, SP], F32,
                                                name=f"er_{h}_{i}_{j}",
                                                tag="er")
                                nc.scalar.activation(out=er, in_=ps, func=EXP)
                                em = ph2em.tile([128, SP], F32R,
                                                name=f"em_{h}_{i}_{j}",
                                                tag=f"em_{i}")
                                nc.vector.tensor_mul(em, er, m_sb[i])
                                em_sb.append(em)
                                nc.sync.dma_start(
                                    out=attnT[h, i * 128:(i + 1) * 128,
                                              j * SP:(j + 1) * SP],
                                    in_=em.bitcast(F32))
                            av = ph2pav.tile([128, SP], F32,
                                             name=f"av_{h}_{j}", tag="ps_av")
                            for i in range(SKT):
                                nc.tensor.matmul(
                                    av[0:65, :],
                                    V1[i][:, h * 65:(h + 1) * 65],
                                    em_sb[i],
                                    start=(i == 0), stop=(i == SKT - 1))
                            rs = ph2bc.tile([1, SP], F32,
                                            name=f"rs_{h}_{j}", tag="rs")
                            nc.vector.tensor_copy(rs, av[64:65, :])
                            nc.sync.dma_start(
                                out=rsums[h:h + 1, j * SP:(j + 1) * SP],
                                in_=rs[0:1, :])
                            bcr = ph2bc.tile([128, SP], F32,
                                             name=f"bcr_{h}_{j}", tag="bcr")
                            nc.gpsimd.partition_broadcast(bcr, rs)
                            bci = ph2bc.tile([128, SP], F32,
                                             name=f"bci_{h}_{j}", tag="bci")
                            nc.vector.reciprocal(bci[0:64, :], bcr[0:64, :])
                            nc.vector.tensor_mul(
                                combT[ht][ho:ho + 64, j * SP:(j + 1) * SP],
                                av[0:64, :], bci[0:64, :])

            # ---------------- Phase 3: output projection ----------------
            with tc.tile_pool(name="ph3w", bufs=1) as ph3w, \
                 tc.tile_pool(name="ph3o", bufs=4) as ph3o, \
                 tc.tile_pool(name="ph3ps", bufs=2, space="PSUM") as ph3ps:
                wo_sb = []
                for k in range(4):
                    wt = ph3w.tile([128, D], F32R, name=f"wo_{k}", tag=f"wo_{k}")
                    nc.sync.dma_start(out=wt, in_=woT[k * 128:(k + 1) * 128, :])
                    wo_sb.append(wt)
                for s in range(SKT):
                    for n2 in range(2):
                        ps = ph3ps.tile([128, 512], F32,
                                        name=f"pso_{s}_{n2}", tag="ps_o")
                        for k in range(4):
                            nc.tensor.matmul(
                                ps,
                                combT[k][:, s * 128:(s + 1) * 128],
                                wo_sb[k][:, n2 * 512:(n2 + 1) * 512],
                                start=(k == 0), stop=(k == 3))
                        ot = ph3o.tile([128, 512], F32,
                                       name=f"ot_{s}_{n2}", tag="ot")
                        nc.vector.tensor_copy(ot, ps)
                        nc.sync.dma_start(
                            out=outp[s * 128:(s + 1) * 128,
                                     n2 * 512:(n2 + 1) * 512],
                            in_=ot)
    nc.compile()
    return nc


def kernel(q, k, v, mask, wq, wk, wv, wo, bq, bk, bv, bo):
    q = np.asarray(q, dtype=np.float32)
    k = np.asarray(k, dtype=np.float32)
    v = np.asarray(v, dtype=np.float32)
    mask = np.asarray(mask)
    wq = np.asarray(wq, dtype=np.float32)
    wk = np.asarray(wk, dtype=np.float32)
    wv = np.asarray(wv, dtype=np.float32)
    wo = np.asarray(wo, dtype=np.float32)
    bq = np.asarray(bq, dtype=np.float32)
    bk = np.asarray(bk, dtype=np.float32)
    bv = np.asarray(bv, dtype=np.float32)
    bo = np.asarray(bo, dtype=np.float32)

    if _CACHED_NC[0] is None:
        _CACHED_NC[0] = _build_nc()
    nc = _CACHED_NC[0]

    ones_arr = np.ones((128, 512), np.float32)
    B = q.shape[0]

    maskTs = {}
    qTs, kTs, vTs = {}, {}, {}
    def _prep_b(b):
        import ml_dtypes
        maskTs[b] = np.ascontiguousarray(mask[b].T).astype(ml_dtypes.bfloat16)
        qTs[b] = np.ascontiguousarray(q[b].T)
        kTs[b] = np.ascontiguousarray(k[b].T)
        vTs[b] = np.ascontiguousarray(v[b].T)
    with ThreadPoolExecutor(max_workers=8) as ex:
        list(ex.map(_prep_b, range(B)))

    in_maps = []
    for c in range(8):
        b, g = c // 2, c % 2
        gs = slice(g * DKH, (g + 1) * DKH)
        in_maps.append({
            "qT": qTs[b], "kT": kTs[b], "vT": vTs[b],
            "maskT": maskTs[b],
            "wqT": np.ascontiguousarray((wq[gs, :] / 8.0).T),
            "wkT": np.ascontiguousarray(wk[gs, :].T),
            "wvT": np.ascontiguousarray(wv[gs, :].T),
            "woT": np.ascontiguousarray(wo[:, gs].T),
            "bq": (bq[gs] / 8.0).reshape(1, DKH).copy(),
            "bk": bk[gs].reshape(1, DKH).copy(),
            "bv": bv[gs].reshape(1, DKH).copy(),
            "ones_d": ones_arr,
        })

    trace = os.environ.get("TRN_MHA_TRACE", "0") == "1"
    res = run_bass_kernel_spmd(nc, in_maps, core_ids=list(range(8)),
                               trace=trace)
    kernel._bass_results = res

    out = np.empty((B, S, D), np.float32)
    for b in range(B):
        out[b] = res.results[2 * b]["outp"] + res.results[2 * b + 1]["outp"] + bo

    attn = np.empty((B, 16, S, S), np.float32)
    def _tp(task):
        c, h = task
        b, g = c // 2, c % 2
        recip = (1.0 / res.results[c]["rsums"][h]).astype(np.float32)
        np.multiply(res.results[c]["attnT"][h].T, recip[:, None],
                    out=attn[b, g * NH + h])
    with ThreadPoolExecutor(max_workers=16) as ex:
        list(ex.map(_tp, [(c, h) for c in range(8) for h in range(NH)]))

    return out, attn
